# revision 1
# baseline (speedup 1.0000x reference)
"""BitLinear (ternary weight quantization + linear) on 8 TRN2 NeuronCores.

y = x @ w_eff.T with w_eff = clip(round(w/scale), -1, 1) * scale,
scale = clamp(mean |w| per row, 1e-5).

Sharding: column-parallel — weight rows (out_features) split 8 ways; each
core computes y[:, shard] for the full x; host concatenates. Quantization
is per-output-row, so it is fully local to a shard.

Matmul runs in fp32r (TF32-like, 11-bit mantissa, full PE rate on TRN2);
measured end-to-end error vs the fp32 reference is ~2e-4 absmax-relative.

Per-core dataflow:
  W phase: for each 128-row chunk of the weight shard, compute the row
  scale (|w| row-sum fused into the Abs activation), build
  w_eff = (w > scale/2)*scale - (w < -scale/2)*scale on the DVE (the
  strict > matches round-half-even semantics of round(w/scale) at the
  0.5 boundary), round to fp32r, PE-transpose, and keep w_eff^T
  resident in SBUF (fp32r, 8 MB).
  X phase: stream 64 row-tiles of x; round to fp32r on the scalar
  engine, PE-transpose into [d_in, row] layout (4 transposes batched
  per PSUM bank, evicted into per-k-group sub-tiles so matmuls start
  early), then 2x16 accumulating N=512 matmuls per tile against the
  resident w_eff^T; evict PSUM via the scalar engine and DMA out.
"""

import numpy as np

import concourse.bass as bass
import concourse.mybir as mybir
import concourse.tile as tile
from concourse import bacc
from concourse.bass_utils import run_bass_kernel_spmd
from concourse.masks import make_identity

F32 = mybir.dt.float32
F32R = mybir.dt.float32r

# Problem shape (hardcoded per contract)
B, S, D_IN, D_OUT = 4, 2048, 2048, 8192
NCORES = 8
R = B * S                 # 8192 rows of x
O = D_OUT // NCORES       # 1024 out features per core
K_SUB = D_IN // 128       # 16 contraction sub-tiles
M_TILES = R // 128        # 64 row tiles
O_TILES = O // 128        # 8 weight row-tiles per core
N_SLICE = 512             # psum bank width (fp32)
N_SLICES = O // N_SLICE   # 2
TGRP = 4                  # transposes batched per psum bank


def _build():
    nc = bacc.Bacc(None, target_bir_lowering=False)

    x_d = nc.dram_tensor("x", [R, D_IN], F32, kind="ExternalInput")
    w_d = nc.dram_tensor("w", [O, D_IN], F32, kind="ExternalInput")
    y_d = nc.dram_tensor("y", [R, O], F32, kind="ExternalOutput")

    with tile.TileContext(nc) as tc:
        with (
            tc.tile_pool(name="const", bufs=1) as const,
            tc.tile_pool(name="wt", bufs=1) as wtp,
            tc.tile_pool(name="ws", bufs=1) as ws,
            tc.tile_pool(name="xs", bufs=3) as xs,
            tc.tile_pool(name="ys", bufs=3) as ysp,
            tc.tile_pool(name="ps", bufs=3, space="PSUM") as ps,
            tc.tile_pool(name="ymm", bufs=4, space="PSUM") as ymm,
        ):
            ident_f = const.tile([128, 128], F32)
            make_identity(nc, ident_f[:])
            ident = const.tile([128, 128], F32R)
            nc.vector.tensor_copy(ident[:], ident_f[:])

            # W^T resident in SBUF, one tile per n-slice:
            # wts[n][:, k, o'] = w_eff^T[i_sub, k, n*512 + o']
            wts = [
                wtp.tile([128, K_SUB, N_SLICE], F32R, name=f"wt{n}")
                for n in range(N_SLICES)
            ]

            def w_chunk(a):
                """Quantize + transpose weight rows a*128..(a+1)*128."""
                w_in = ws.tile([128, D_IN], F32, tag="w_in", bufs=2,
                               name=f"w_in_{a}")
                nc.sync.dma_start(w_in[:], w_d[a * 128 : (a + 1) * 128, :])

                # |w| row-sum fused into the Abs activation; the abs
                # values land in the buffer later reused for `neg`
                absdump = ws.tile([128, D_IN], F32, tag="w_neg",
                                  name=f"absdump_{a}")
                ssum = ws.tile([128, 1], F32, tag="w_sum", name=f"ssum_{a}")
                nc.scalar.activation(
                    absdump[:], w_in[:],
                    mybir.ActivationFunctionType.Abs,
                    accum_out=ssum[:],
                )
                scale = ws.tile([128, 1], F32, tag="w_scale",
                                name=f"scale_{a}")
                nc.vector.tensor_scalar(
                    out=scale[:], in0=ssum[:], scalar1=1.0 / D_IN,
                    scalar2=1e-5, op0=mybir.AluOpType.mult,
                    op1=mybir.AluOpType.max,
                )
                hpos = ws.tile([128, 1], F32, tag="w_hpos", name=f"hp_{a}")
                hneg = ws.tile([128, 1], F32, tag="w_hneg", name=f"hn_{a}")
                nc.vector.tensor_scalar_mul(hpos[:], scale[:], 0.5)
                nc.vector.tensor_scalar_mul(hneg[:], scale[:], -0.5)

                # (w > 0.5*scale)*scale - (w < -0.5*scale)*scale
                pos = ws.tile([128, D_IN], F32, tag="w_pos", name=f"pos_{a}")
                nc.vector.tensor_scalar(
                    out=pos[:], in0=w_in[:], scalar1=hpos[:], scalar2=scale[:],
                    op0=mybir.AluOpType.is_gt, op1=mybir.AluOpType.mult,
                )
                neg = ws.tile([128, D_IN], F32, tag="w_neg", name=f"neg_{a}")
                nc.vector.tensor_scalar(
                    out=neg[:], in0=w_in[:], scalar1=hneg[:], scalar2=scale[:],
                    op0=mybir.AluOpType.is_lt, op1=mybir.AluOpType.mult,
                )
                weff = ws.tile([128, D_IN], F32R, tag="w_eff",
                               name=f"weff_{a}")
                nc.vector.tensor_sub(weff[:], pos[:], neg[:])

                n_idx, o_off = divmod(a * 128, N_SLICE)
                for kg in range(K_SUB // TGRP):
                    pt = ps.tile([128, TGRP * 128], F32, tag="wtps", bufs=2,
                                 name=f"wpt_{a}_{kg}")
                    for j in range(TGRP):
                        k = kg * TGRP + j
                        nc.tensor.transpose(
                            pt[:, j * 128 : (j + 1) * 128].bitcast(F32R),
                            weff[:, k * 128 : (k + 1) * 128],
                            ident[:],
                        )
                    half = TGRP // 2
                    dst = wts[n_idx][:, kg * TGRP : (kg + 1) * TGRP,
                                     o_off : o_off + 128]
                    src = pt[:].rearrange("p (g c) -> p g c", g=TGRP)
                    # both halves on ACT: the W-phase cycle is DVE-paced
                    # (pos+neg+sub), ACT has slack there
                    nc.scalar.copy(dst[:, :half], src[:, :half])
                    nc.scalar.copy(dst[:, half:], src[:, half:])

            def x_stage(m):
                """Load/round/transpose x row-tile m; returns x_t sub-tiles."""
                x_in = xs.tile([128, D_IN], F32, tag="x_in", bufs=2,
                               name=f"x_in_{m}")
                nc.sync.dma_start(x_in[:], x_d[m * 128 : (m + 1) * 128, :])
                x_r = xs.tile([128, D_IN], F32R, tag="x_r", bufs=2,
                              name=f"x_r_{m}")
                nc.scalar.copy(x_r[:], x_in[:])

                x_ts = []
                for kg in range(K_SUB // TGRP):
                    pt = ps.tile([128, TGRP * 128], F32, tag="xtps", bufs=3,
                                 name=f"xpt_{m}_{kg}")
                    for j in range(TGRP):
                        k = kg * TGRP + j
                        nc.tensor.transpose(
                            pt[:, j * 128 : (j + 1) * 128].bitcast(F32R),
                            x_r[:, k * 128 : (k + 1) * 128],
                            ident[:],
                        )
                    x_t = xs.tile(
                        [128, TGRP, 128], F32R, tag=f"x_t{kg}", bufs=5,
                        name=f"x_t{kg}_{m}",
                    )
                    nc.vector.tensor_copy(x_t[:], pt[:])
                    x_ts.append(x_t)
                return x_ts

            def mm_group(m, n, x_ts):
                """One accumulation group + eviction + half-row store."""
                acc = ymm.tile([128, N_SLICE], F32, tag="y_ps",
                               name=f"acc{n}_{m}", bufs=3)
                for k in range(K_SUB):
                    nc.tensor.matmul(
                        acc[:],
                        x_ts[k // TGRP][:, k % TGRP, :],
                        wts[n][:, k, :],
                        start=(k == 0),
                        stop=(k == K_SUB - 1),
                    )
                y_sb = ysp.tile([128, N_SLICE], F32, tag=f"y_sb{n}",
                                name=f"y_sb{n}_{m}", bufs=3)
                nc.scalar.copy(y_sb[:], acc[:])
                nc.sync.dma_start(
                    y_d[m * 128 : (m + 1) * 128,
                        n * N_SLICE : (n + 1) * N_SLICE],
                    y_sb[:],
                )

            # Emission schedule: W chunks 0-3 produce wts[0]; then x tiles
            # 0-3 run their n=0 groups interleaved with W chunks 4-7 (which
            # produce wts[1]) so the PE never starves during W quant; then
            # the n=1 groups catch up; then steady state.
            NPRE = O_TILES - N_SLICE // 128  # 4
            pre_xts = []
            for a in range(4):
                w_chunk(a)
                pre_xts.append(x_stage(a))
            for m in range(NPRE):
                mm_group(m, 0, pre_xts[m])
                if 4 + m < O_TILES:
                    w_chunk(4 + m)
            for m in range(NPRE):
                mm_group(m, 1, pre_xts[m])
            # Steady state, software-pipelined: the next tile's transposes
            # are emitted between the current tile's two matmul groups so
            # the x_t evictions land before the PE needs them.
            prev_m, prev_xts = NPRE, x_stage(NPRE)
            for m in range(NPRE + 1, M_TILES):
                mm_group(prev_m, 0, prev_xts)
                cur_xts = x_stage(m)
                mm_group(prev_m, 1, prev_xts)
                prev_m, prev_xts = m, cur_xts
            mm_group(prev_m, 0, prev_xts)
            mm_group(prev_m, 1, prev_xts)

    nc.compile()
    return nc


_NC_CACHE = None


def _get_nc():
    global _NC_CACHE
    if _NC_CACHE is None:
        _NC_CACHE = _build()
    return _NC_CACHE


def kernel(x: np.ndarray, weight: np.ndarray, _trace: bool = False):
    assert x.shape == (B, S, D_IN) and weight.shape == (D_OUT, D_IN)
    x_flat = np.ascontiguousarray(x.reshape(R, D_IN), dtype=np.float32)
    in_maps = [
        {
            "x": x_flat,
            "w": np.ascontiguousarray(
                weight[c * O : (c + 1) * O], dtype=np.float32
            ),
        }
        for c in range(NCORES)
    ]
    nc = _get_nc()
    res = run_bass_kernel_spmd(
        nc, in_maps, core_ids=list(range(NCORES)), trace=_trace
    )
    y = np.concatenate([res.results[c]["y"] for c in range(NCORES)], axis=1)
    out = y.reshape(B, S, D_OUT)
    if _trace:
        return out, res
    return out



# revision 2
# speedup vs baseline: 1.1839x; 1.1839x over previous
"""BitLinear (ternary weight quantization + linear) on 8 TRN2 NeuronCores.

y = x @ w_eff.T with w_eff = clip(round(w/scale), -1, 1) * scale,
scale = clamp(mean |w| per row, 1e-5).

Sharding: column-parallel — weight rows (out_features) split 8 ways; each
core computes y[:, shard] for the full x; host concatenates. Quantization
is per-output-row, so it is fully local to a shard.

v2 dataflow (vs the transpose-on-device baseline):
  * The host pre-permutes x into [16 chunks][128 k_in][16 k_sub][512 rows]
    so every x tile lands in SBUF already in stationary (k-major) layout —
    the device does ZERO x preprocessing (the baseline spent ~86us of PE
    time on 1024 PE transposes plus DVE/ACT copies for this).
  * The matmul runs in bf16: w_eff is ternary*scale (exact in bf16 up to a
    coherent 0.4% per-row scale rounding), x is cast fp32->bf16 in-flight
    by the SWDGE DMA. bf16 enables fast weight load so the per-matmul
    LDWEIGHTS (~187ns in fp32r) hides completely under the 213ns matmul.
  * W phase keeps the baseline's quantization math bit-identical (Abs
    row-sum on ACT, scale clamp, is_gt/is_lt ternary build on DVE): the
    jax reference's round() at the 0.5 boundary is reproduced exactly —
    a single flipped ternary weight costs 1.35e-2 absmax error, 2/3 of
    the 2e-2 budget.
  * Warm-up matmuls at kernel start bring the PE HAM clock gate to 8/8
    before the real matmul stream begins.

Per-core steady state: 16 row-chunks of 512; per chunk one 4MiB cast-DMA
in, then 8 accumulation groups (4 m-subtiles x 2 n-slices) of 16 matmuls
[128x128]@[128x512], ACT eviction, 256KB y DMA out per group.
"""

import numpy as np

import concourse.bass as bass
import concourse.mybir as mybir
import concourse.tile as tile
from concourse import bacc
from concourse.bass_utils import run_bass_kernel_spmd
from concourse.masks import make_identity

F32 = mybir.dt.float32
F32R = mybir.dt.float32r
BF16 = mybir.dt.bfloat16

# Problem shape (hardcoded per contract)
B, S, D_IN, D_OUT = 4, 2048, 2048, 8192
NCORES = 8
R = B * S                 # 8192 rows of x
O = D_OUT // NCORES       # 1024 out features per core
K_SUB = D_IN // 128       # 16 contraction sub-tiles
O_TILES = O // 128        # 8 weight row-tiles per core
N_SLICE = 512             # psum bank width (fp32)
N_SLICES = O // N_SLICE   # 2
TGRP = 4                  # transposes batched per psum bank
RCHUNK = 512              # x rows per streamed chunk
NCHUNK = R // RCHUNK      # 16
MSUB = RCHUNK // 128      # 4
N_WARM = 20               # HAM warm-up matmuls


def _build():
    nc = bacc.Bacc(None, target_bir_lowering=False)

    # x: host-permuted [chunk, k_in, k_sub, row] (full x, same on all cores)
    x_d = nc.dram_tensor("x", [NCHUNK, 128, K_SUB, RCHUNK], F32,
                         kind="ExternalInput")
    w_d = nc.dram_tensor("w", [O, D_IN], F32, kind="ExternalInput")
    y_d = nc.dram_tensor("y", [R, O], F32, kind="ExternalOutput")

    with tile.TileContext(nc) as tc:
        with (
            tc.tile_pool(name="const", bufs=1) as const,
            tc.tile_pool(name="wt", bufs=1) as wtp,
            tc.tile_pool(name="ws", bufs=1) as ws,
            tc.tile_pool(name="xs", bufs=1) as xs,
            tc.tile_pool(name="ys", bufs=1) as ysp,
            tc.tile_pool(name="ps", bufs=2, space="PSUM") as ps,
            tc.tile_pool(name="ymm", bufs=1, space="PSUM") as ymm,
        ):
            # HAM warm-up: keep the PE busy with throwaway matmuls during
            # the W-phase lead-in so the clock gate is at 8/8 when the
            # real stream starts. (PE transposes don't count as HAM-busy.)
            dummy = const.tile([128, N_SLICE], BF16)
            nc.vector.memset(dummy[:], 0.0)
            wacc = ymm.tile([128, N_SLICE], F32, tag="warm", bufs=1)
            for _ in range(N_WARM):
                nc.tensor.matmul(dummy_out := wacc[:], dummy[:, :128],
                                 dummy[:], start=True, stop=True)

            ident_f = const.tile([128, 128], F32)
            make_identity(nc, ident_f[:])
            ident = const.tile([128, 128], F32R)
            nc.vector.tensor_copy(ident[:], ident_f[:])

            # W^T resident in SBUF (bf16), one tile per n-slice:
            # wts[n][:, k, o'] = w_eff^T[k_in, k_sub, n*512 + o']
            wts = [
                wtp.tile([128, K_SUB, N_SLICE], BF16, name=f"wt{n}")
                for n in range(N_SLICES)
            ]

            def w_chunk(a):
                """Quantize + transpose weight rows a*128..(a+1)*128.

                Math is bit-identical to the baseline (matches the jax
                reference's round-half behavior at the 0.5 boundary); only
                the final PSUM->SBUF eviction casts to bf16.
                """
                w_in = ws.tile([128, D_IN], F32, tag="w_in", bufs=2,
                               name=f"w_in_{a}")
                nc.sync.dma_start(w_in[:], w_d[a * 128 : (a + 1) * 128, :])

                absdump = ws.tile([128, D_IN], F32, tag="w_neg",
                                  name=f"absdump_{a}")
                ssum = ws.tile([128, 1], F32, tag="w_sum", name=f"ssum_{a}")
                nc.scalar.activation(
                    absdump[:], w_in[:],
                    mybir.ActivationFunctionType.Abs,
                    accum_out=ssum[:],
                )
                scale = ws.tile([128, 1], F32, tag="w_scale",
                                name=f"scale_{a}")
                nc.vector.tensor_scalar(
                    out=scale[:], in0=ssum[:], scalar1=1.0 / D_IN,
                    scalar2=1e-5, op0=mybir.AluOpType.mult,
                    op1=mybir.AluOpType.max,
                )
                hpos = ws.tile([128, 1], F32, tag="w_hpos", name=f"hp_{a}")
                hneg = ws.tile([128, 1], F32, tag="w_hneg", name=f"hn_{a}")
                nc.vector.tensor_scalar_mul(hpos[:], scale[:], 0.5)
                nc.vector.tensor_scalar_mul(hneg[:], scale[:], -0.5)

                # (w > 0.5*scale)*scale - (w < -0.5*scale)*scale
                pos = ws.tile([128, D_IN], F32, tag="w_pos", name=f"pos_{a}")
                nc.vector.tensor_scalar(
                    out=pos[:], in0=w_in[:], scalar1=hpos[:], scalar2=scale[:],
                    op0=mybir.AluOpType.is_gt, op1=mybir.AluOpType.mult,
                )
                neg = ws.tile([128, D_IN], F32, tag="w_neg", name=f"neg_{a}")
                nc.vector.tensor_scalar(
                    out=neg[:], in0=w_in[:], scalar1=hneg[:], scalar2=scale[:],
                    op0=mybir.AluOpType.is_lt, op1=mybir.AluOpType.mult,
                )
                weff = ws.tile([128, D_IN], F32R, tag="w_eff",
                               name=f"weff_{a}")
                nc.vector.tensor_sub(weff[:], pos[:], neg[:])

                n_idx, o_off = divmod(a * 128, N_SLICE)
                for kg in range(K_SUB // TGRP):
                    pt = ps.tile([128, TGRP * 128], F32, tag="wtps", bufs=2,
                                 name=f"wpt_{a}_{kg}")
                    for j in range(TGRP):
                        k = kg * TGRP + j
                        nc.tensor.transpose(
                            pt[:, j * 128 : (j + 1) * 128].bitcast(F32R),
                            weff[:, k * 128 : (k + 1) * 128],
                            ident[:],
                        )
                    half = TGRP // 2
                    dst = wts[n_idx][:, kg * TGRP : (kg + 1) * TGRP,
                                     o_off : o_off + 128]
                    src = pt[:].rearrange("p (g c) -> p g c", g=TGRP)
                    nc.scalar.copy(dst[:, :half], src[:, :half])
                    nc.scalar.copy(dst[:, half:], src[:, half:])

            def x_load(c):
                """Start the cast-DMA for x chunk c; returns the tile."""
                xk = xs.tile([128, K_SUB, RCHUNK], BF16, tag="xk", bufs=3,
                             name=f"xk_{c}")
                nc.gpsimd.dma_start(xk[:], x_d[c])
                return xk

            def mm_group(c, m, n, xk):
                """One accumulation group + eviction + 256KB y store."""
                acc = ymm.tile([128, N_SLICE], F32, tag="y_ps",
                               name=f"acc_{c}_{m}_{n}", bufs=4)
                lhs = xk[:, :, m * 128 : (m + 1) * 128]
                for k in range(K_SUB):
                    nc.tensor.matmul(
                        acc[:],
                        lhs[:, k, :],
                        wts[n][:, k, :],
                        start=(k == 0),
                        stop=(k == K_SUB - 1),
                    )
                y_sb = ysp.tile([128, N_SLICE], F32, tag="y_sb",
                                name=f"y_sb_{c}_{m}_{n}", bufs=4)
                nc.scalar.copy(y_sb[:], acc[:])
                nc.sync.dma_start(
                    y_d[(c * MSUB + m) * 128 : (c * MSUB + m + 1) * 128,
                        n * N_SLICE : (n + 1) * N_SLICE],
                    y_sb[:],
                )

            # Emission schedule: W chunks 0-3 produce wts[0]; x chunks 0-1
            # prefetch under them. Chunk 0's n=0 groups run while W chunks
            # 4-7 (producing wts[1]) finish on DVE/ACT, then its n=1
            # groups; chunks 1+ run both n per m-subtile, with the next
            # chunk's DMA issued one chunk ahead.
            for a in range(4):
                w_chunk(a)
            xk_cur = x_load(0)
            xk_next = x_load(1)
            for m in range(MSUB):
                mm_group(0, m, 0, xk_cur)
                if m < 4:
                    w_chunk(4 + m)
            for m in range(MSUB):
                mm_group(0, m, 1, xk_cur)
            for c in range(1, NCHUNK):
                xk_cur = xk_next
                if c + 1 < NCHUNK:
                    xk_next = x_load(c + 1)
                for m in range(MSUB):
                    mm_group(c, m, 0, xk_cur)
                    mm_group(c, m, 1, xk_cur)

    nc.compile()
    return nc


_NC_CACHE = None


def _get_nc():
    global _NC_CACHE
    if _NC_CACHE is None:
        _NC_CACHE = _build()
    return _NC_CACHE


def kernel(x: np.ndarray, weight: np.ndarray, _trace: bool = False):
    assert x.shape == (B, S, D_IN) and weight.shape == (D_OUT, D_IN)
    # Host layout prep: [chunk, k_in, k_sub, row] so each chunk DMAs in as
    # ready-to-use stationary tiles (k on partitions), 32KB contiguous per
    # partition.
    x_flat = np.asarray(x, dtype=np.float32).reshape(R, D_IN)
    xr = np.ascontiguousarray(
        x_flat.reshape(NCHUNK, RCHUNK, K_SUB, 128).transpose(0, 3, 2, 1)
    )
    in_maps = [
        {
            "x": xr,
            "w": np.ascontiguousarray(
                weight[c * O : (c + 1) * O], dtype=np.float32
            ),
        }
        for c in range(NCORES)
    ]
    nc = _get_nc()
    res = run_bass_kernel_spmd(
        nc, in_maps, core_ids=list(range(NCORES)), trace=_trace
    )
    y = np.concatenate([res.results[c]["y"] for c in range(NCORES)], axis=1)
    out = y.reshape(B, S, D_OUT)
    if _trace:
        return out, res
    return out


# revision 9
# speedup vs baseline: 1.2612x; 1.0653x over previous
"""BitLinear (ternary weight quantization + linear) on 8 TRN2 NeuronCores.

y = x @ w_eff.T with w_eff = clip(round(w/scale), -1, 1) * scale,
scale = clamp(mean |w| per row, 1e-5).

Sharding: column-parallel — weight rows (out_features) split 8 ways; each
core computes y[:, shard] for the full x; host concatenates. Quantization
is per-output-row, so it is fully local to a shard.

v2 dataflow (vs the transpose-on-device baseline):
  * The host pre-permutes x into [16 chunks][128 k_in][16 k_sub][512 rows]
    so every x tile lands in SBUF already in stationary (k-major) layout —
    the device does ZERO x preprocessing (the baseline spent ~86us of PE
    time on 1024 PE transposes plus DVE/ACT copies for this).
  * The matmul runs in bf16: w_eff is ternary*scale (exact in bf16 up to a
    coherent 0.4% per-row scale rounding), x is cast fp32->bf16 in-flight
    by the SWDGE DMA. bf16 enables fast weight load so the per-matmul
    LDWEIGHTS (~187ns in fp32r) hides completely under the 213ns matmul.
  * W phase keeps the baseline's quantization math bit-identical (Abs
    row-sum on ACT, scale clamp, is_gt/is_lt ternary build on DVE): the
    jax reference's round() at the 0.5 boundary is reproduced exactly —
    a single flipped ternary weight costs 1.35e-2 absmax error, 2/3 of
    the 2e-2 budget.
  * Warm-up matmuls at kernel start bring the PE HAM clock gate to 8/8
    before the real matmul stream begins.

Per-core steady state: 16 row-chunks of 512; per chunk one 4MiB cast-DMA
in, then 8 accumulation groups (4 m-subtiles x 2 n-slices) of 16 matmuls
[128x128]@[128x512], ACT eviction, 256KB y DMA out per group.
"""

import numpy as np

import concourse.bass as bass
import concourse.mybir as mybir
import concourse.tile as tile
from concourse import bacc
from concourse.bass_utils import run_bass_kernel_spmd
from concourse.masks import make_identity

F32 = mybir.dt.float32
F32R = mybir.dt.float32r
BF16 = mybir.dt.bfloat16

# Problem shape (hardcoded per contract)
B, S, D_IN, D_OUT = 4, 2048, 2048, 8192
NCORES = 8
R = B * S                 # 8192 rows of x
O = D_OUT // NCORES       # 1024 out features per core
K_SUB = D_IN // 128       # 16 contraction sub-tiles
O_TILES = O // 128        # 8 weight row-tiles per core
N_SLICE = 512             # psum bank width (fp32)
N_SLICES = O // N_SLICE   # 2
TGRP = 4                  # transposes batched per psum bank
RCHUNK = 512              # x rows per streamed chunk
NCHUNK = R // RCHUNK      # 16
MSUB = RCHUNK // 128      # 4
N_WARM = 20               # HAM warm-up matmuls


def _build():
    nc = bacc.Bacc(None, target_bir_lowering=False)

    # x: host-permuted [chunk, k_in, msub, k_sub, row] (full x, same on all
    # cores); one 1MiB sub-DMA per (chunk, msub) keeps dependencies fine-
    # grained so the first matmul group only waits for 1MiB of x.
    x_d = nc.dram_tensor("x", [NCHUNK, 128, MSUB, K_SUB, 128], F32,
                         kind="ExternalInput")
    w_d = nc.dram_tensor("w", [O, D_IN], F32, kind="ExternalInput")
    y_d = nc.dram_tensor("y", [R, O], F32, kind="ExternalOutput")

    with tile.TileContext(nc) as tc:
        with (
            tc.tile_pool(name="const", bufs=1) as const,
            tc.tile_pool(name="wt", bufs=1) as wtp,
            tc.tile_pool(name="ws", bufs=1) as ws,
            tc.tile_pool(name="xs", bufs=1) as xs,
            tc.tile_pool(name="ys", bufs=1) as ysp,
            tc.tile_pool(name="ps", bufs=2, space="PSUM") as ps,
            tc.tile_pool(name="ymm", bufs=1, space="PSUM") as ymm,
        ):
            # HAM warm-up: keep the PE busy with throwaway matmuls during
            # the W-phase lead-in so the clock gate is at 8/8 when the
            # real stream starts. (PE transposes don't count as HAM-busy.)
            dummy = const.tile([128, N_SLICE], BF16)
            nc.vector.memset(dummy[:], 0.0)
            wacc = ymm.tile([128, N_SLICE], F32, tag="warm", bufs=1)

            def warmup(n):
                for _ in range(n):
                    nc.tensor.matmul(wacc[:], dummy[:, :128], dummy[:],
                                     start=True, stop=True)

            ident_f = const.tile([128, 128], F32)
            make_identity(nc, ident_f[:])
            ident = const.tile([128, 128], F32R)
            nc.vector.tensor_copy(ident[:], ident_f[:])

            # W^T resident in SBUF (bf16), one tile per n-slice:
            # wts[n][:, k, o'] = w_eff^T[k_in, k_sub, n*512 + o']
            wts = [
                wtp.tile([128, K_SUB, N_SLICE], BF16, name=f"wt{n}")
                for n in range(N_SLICES)
            ]

            def w_chunk(a):
                """Quantize + transpose weight rows a*128..(a+1)*128.

                Math is bit-identical to the baseline (matches the jax
                reference's round-half behavior at the 0.5 boundary); only
                the final PSUM->SBUF eviction casts to bf16.
                """
                w_in = ws.tile([128, D_IN], F32, tag="w_in", bufs=3,
                               name=f"w_in_{a}")
                nc.sync.dma_start(w_in[:], w_d[a * 128 : (a + 1) * 128, :])

                absdump = ws.tile([128, D_IN], F32, tag="w_neg",
                                  name=f"absdump_{a}")
                ssum = ws.tile([128, 1], F32, tag="w_sum", name=f"ssum_{a}")
                nc.scalar.activation(
                    absdump[:], w_in[:],
                    mybir.ActivationFunctionType.Abs,
                    accum_out=ssum[:],
                )
                scale = ws.tile([128, 1], F32, tag="w_scale",
                                name=f"scale_{a}")
                nc.vector.tensor_scalar(
                    out=scale[:], in0=ssum[:], scalar1=1.0 / D_IN,
                    scalar2=1e-5, op0=mybir.AluOpType.mult,
                    op1=mybir.AluOpType.max,
                )
                hpos = ws.tile([128, 1], F32, tag="w_hpos", name=f"hp_{a}")
                hneg = ws.tile([128, 1], F32, tag="w_hneg", name=f"hn_{a}")
                nc.vector.tensor_scalar_mul(hpos[:], scale[:], 0.5)
                nc.vector.tensor_scalar_mul(hneg[:], scale[:], -0.5)

                # (w > 0.5*scale)*scale - (w < -0.5*scale)*scale
                pos = ws.tile([128, D_IN], F32, tag="w_pos", name=f"pos_{a}")
                nc.vector.tensor_scalar(
                    out=pos[:], in0=w_in[:], scalar1=hpos[:], scalar2=scale[:],
                    op0=mybir.AluOpType.is_gt, op1=mybir.AluOpType.mult,
                )
                neg = ws.tile([128, D_IN], F32, tag="w_neg", name=f"neg_{a}")
                nc.vector.tensor_scalar(
                    out=neg[:], in0=w_in[:], scalar1=hneg[:], scalar2=scale[:],
                    op0=mybir.AluOpType.is_lt, op1=mybir.AluOpType.mult,
                )
                weff = ws.tile([128, D_IN], F32R, tag="w_eff",
                               name=f"weff_{a}")
                nc.vector.tensor_sub(weff[:], pos[:], neg[:])

                n_idx, o_off = divmod(a * 128, N_SLICE)
                for kg in range(K_SUB // TGRP):
                    pt = ps.tile([128, TGRP * 128], F32, tag="wtps", bufs=2,
                                 name=f"wpt_{a}_{kg}")
                    for j in range(TGRP):
                        k = kg * TGRP + j
                        nc.tensor.transpose(
                            pt[:, j * 128 : (j + 1) * 128].bitcast(F32R),
                            weff[:, k * 128 : (k + 1) * 128],
                            ident[:],
                        )
                    half = TGRP // 2
                    dst = wts[n_idx][:, kg * TGRP : (kg + 1) * TGRP,
                                     o_off : o_off + 128]
                    src = pt[:].rearrange("p (g c) -> p g c", g=TGRP)
                    nc.scalar.copy(dst[:, :half], src[:, :half])
                    nc.scalar.copy(dst[:, half:], src[:, half:])

            def x_load(c):
                """Start 4 per-msub cast-DMAs for x chunk c (1MiB each)."""
                tiles = []
                for m in range(MSUB):
                    xm = xs.tile([128, K_SUB, 128], BF16, tag=f"x{m}",
                                 bufs=4, name=f"x{m}_{c}")
                    nc.gpsimd.dma_start(xm[:], x_d[c, :, m])
                    tiles.append(xm)
                return tiles

            def mm_group(c, m, n, xk):
                """One accumulation group + eviction + 256KB y store."""
                acc = ymm.tile([128, N_SLICE], F32, tag="y_ps",
                               name=f"acc_{c}_{m}_{n}", bufs=4)
                lhs = xk[m]
                for k in range(K_SUB):
                    nc.tensor.matmul(
                        acc[:],
                        lhs[:, k, :],
                        wts[n][:, k, :],
                        start=(k == 0),
                        stop=(k == K_SUB - 1),
                    )
                y_sb = ysp.tile([128, N_SLICE], F32, tag="y_sb",
                                name=f"y_sb_{c}_{m}_{n}", bufs=4)
                nc.scalar.copy(y_sb[:], acc[:])
                nc.sync.dma_start(
                    y_d[(c * MSUB + m) * 128 : (c * MSUB + m + 1) * 128,
                        n * N_SLICE : (n + 1) * N_SLICE],
                    y_sb[:],
                )

            # Emission schedule. Lead-in critical path is HBM: the first
            # group needs w chunks 0-3 (4MiB -> wts[0]) plus 1MiB of x, so
            # x sub-DMAs are issued first and warm-up matmuls bridge the
            # PE until data lands. wts[1] (W chunks 4-7, DVE-serial quant)
            # arrives ~30us in, so chunks 0-1 run their n=0 groups before
            # any n=1 group.
            xk0 = x_load(0)
            warmup(12)
            for a in range(4):
                w_chunk(a)
            xk1 = x_load(1)
            warmup(14)
            for m in range(MSUB):
                mm_group(0, m, 0, xk0)
                if m < 4:
                    w_chunk(4 + m)
            xk2 = x_load(2)
            for m in range(MSUB):
                mm_group(1, m, 0, xk1)
            for m in range(MSUB):
                mm_group(0, m, 1, xk0)
            xk3 = x_load(3)
            for m in range(MSUB):
                mm_group(1, m, 1, xk1)
            xk_tiles = {2: xk2, 3: xk3}
            for c in range(2, NCHUNK):
                if c + 2 < NCHUNK:
                    xk_tiles[c + 2] = x_load(c + 2)
                for m in range(MSUB):
                    mm_group(c, m, 0, xk_tiles[c])
                    mm_group(c, m, 1, xk_tiles[c])

    nc.compile()
    return nc


_NC_CACHE = None


def _get_nc():
    global _NC_CACHE
    if _NC_CACHE is None:
        _NC_CACHE = _build()
    return _NC_CACHE


def kernel(x: np.ndarray, weight: np.ndarray, _trace: bool = False):
    assert x.shape == (B, S, D_IN) and weight.shape == (D_OUT, D_IN)
    # Host layout prep: [chunk, k_in, msub, k_sub, row] so each (chunk,
    # msub) DMAs in as ready-to-use stationary tiles (k on partitions),
    # contiguous per partition.
    x_flat = np.asarray(x, dtype=np.float32).reshape(R, D_IN)
    xr = np.ascontiguousarray(
        x_flat.reshape(NCHUNK, MSUB, 128, K_SUB, 128)
        .transpose(0, 4, 1, 3, 2)
    )
    in_maps = [
        {
            "x": xr,
            "w": np.ascontiguousarray(
                weight[c * O : (c + 1) * O], dtype=np.float32
            ),
        }
        for c in range(NCORES)
    ]
    nc = _get_nc()
    res = run_bass_kernel_spmd(
        nc, in_maps, core_ids=list(range(NCORES)), trace=_trace
    )
    y = np.concatenate([res.results[c]["y"] for c in range(NCORES)], axis=1)
    out = y.reshape(B, S, D_OUT)
    if _trace:
        return out, res
    return out
